# revision 9
# baseline (speedup 1.0000x reference)
"""Trainium2 Bass kernel for nn_PhysicsLoss (ionospheric Abel-loss).

Strategy: pure data parallel over batch B=256 -> 8 cores x 32 rows.
Each core computes partial sums of the three losses on-device; the host
does the final (tiny) scalar combine of 8 small partial vectors.

Per-core layout for the dominant (B,N_h,N_f) Abel term, per (b, h-tile):
tiles are [128 h-partitions, 256 f-free].
  DVE : pen   = (threshp <= cummax_col) * 1e18         (tensor_scalar dual)
  DVE : ratio = invf2p * ne_col                        (tensor_scalar)
  DVE : rs2   = pen - ratio                            (tensor_tensor)
  ACT : mu    = exp(-0.5*ln(1 + rs2))  [or abs_rsqrt]  = masked 1/sqrt(1-ratio)
  PE  : hv[:, b] += mu[:, half]^T @ w_col              (h-reduction matmul)

The reflect mask uses the identity cummax_h(ratio >= g) == (cummax_h(ne) >=
g*c*(f2+eps)), so the 3D cummax collapses to a 2D prefix-max computed by
log-doubling shifted maxes. Where keep: ratio < 0.999 automatically
(ne <= cummax < thresh), so no min-clamp is needed and the log argument
stays in [1e-3, 1]; masked entries get arg ~ 1e18 -> mu ~ 1e-9 ~ 0.

NOTE: tensor_tensor_scan, scalar_tensor_tensor (3-input DVE forms) hang on
this HW path, and tensor_tensor_reduce fails at runtime - do not use them.
"""

import os
import numpy as np
from contextlib import ExitStack

import concourse.bass as bass
import concourse.tile as tile
from concourse import bacc, mybir
from concourse.bass_utils import run_bass_kernel_spmd

FP2_CONST = 1.24e4
REFLECT_GUARD = 0.999
LAMBDA_PHY, LAMBDA_MONO, LAMBDA_BG = 1.0, 0.01, 0.1
EPS_MONO = 1e-06

N_CORES = 8
B, N_H, N_F = 256, 512, 256
BL = B // N_CORES          # 32 local batch rows
N_HT = N_H // 128          # 4 h-tiles

F32 = mybir.dt.float32
AF = mybir.ActivationFunctionType
ALU = mybir.AluOpType

_cache = {}
MU_PATH = os.environ.get("KMU", "lnexp")  # "lnexp" | "absrsqrt"


def _build_program():
    nc = bacc.Bacc("TRN2", target_bir_lowering=False, debug=False,
                   num_devices=N_CORES)

    ins = {}
    for name, shape in [
        ("ne", [BL, N_H]), ("ne_t", [N_H, BL]), ("iri", [BL, N_H]),
        ("obs_t", [N_F, BL]), ("mask_t", [N_F, BL]),
        ("invf2p", [128, N_F]), ("threshp", [128, N_F]),
        ("wmat", [128, N_HT]), ("eye32", [BL, BL]),
        ("hbasecol", [128, 1]), ("cepscol", [128, 1]),
    ]:
        ins[name] = nc.dram_tensor(name, shape, F32, kind="ExternalInput").ap()
    out = nc.dram_tensor("out", [128, 8], F32, kind="ExternalOutput").ap()

    with tile.TileContext(nc) as tc, ExitStack() as ctx:
        cpool = ctx.enter_context(tc.tile_pool(name="consts", bufs=1))
        pen_pool = ctx.enter_context(tc.tile_pool(name="pen", bufs=4))
        rat_pool = ctx.enter_context(tc.tile_pool(name="rat", bufs=4))
        rs_pool = ctx.enter_context(tc.tile_pool(name="rs", bufs=4))
        mu_pool = ctx.enter_context(tc.tile_pool(name="mu", bufs=4))
        ppool = ctx.enter_context(
            tc.tile_pool(name="tpsum", bufs=2, space=bass.MemorySpace.PSUM))
        hvpool = ctx.enter_context(
            tc.tile_pool(name="hvpsum", bufs=1, space=bass.MemorySpace.PSUM))

        # ---- load inputs ----
        cne = cpool.tile([BL, N_H], F32, tag="cne")
        cnet = cpool.tile([128, N_HT * BL], F32, tag="cnet")   # [h, ht*32+b]
        ciri = cpool.tile([BL, N_H], F32, tag="ciri")
        cobst = cpool.tile([128, 2 * BL], F32, tag="cobst")    # [f, hf*32+b]
        cmaskt = cpool.tile([128, 2 * BL], F32, tag="cmaskt")
        cinv = cpool.tile([128, N_F], F32, tag="cinv")
        cthr = cpool.tile([128, N_F], F32, tag="cthr")
        cw = cpool.tile([128, N_HT], F32, tag="cw")
        ceye = cpool.tile([BL, BL], F32, tag="ceye")
        chb = cpool.tile([128, 1], F32, tag="chb")
        ceps_t = cpool.tile([128, 1], F32, tag="ceps_t")
        loads = [(cne[:], ins["ne"]), (ciri[:], ins["iri"]),
                 (cinv[:], ins["invf2p"]), (cthr[:], ins["threshp"]),
                 (cw[:], ins["wmat"]), (ceye[:], ins["eye32"]),
                 (chb[:], ins["hbasecol"]), (ceps_t[:], ins["cepscol"])]
        net_d = ins["ne_t"].rearrange("(t p) b -> t p b", p=128)
        for ht in range(N_HT):
            loads.append((cnet[:, ht * BL:(ht + 1) * BL], net_d[ht]))
        obst_d = ins["obs_t"].rearrange("(t p) b -> t p b", p=128)
        maskt_d = ins["mask_t"].rearrange("(t p) b -> t p b", p=128)
        for hf in range(2):
            loads.append((cobst[:, hf * BL:(hf + 1) * BL], obst_d[hf]))
            loads.append((cmaskt[:, hf * BL:(hf + 1) * BL], maskt_d[hf]))
        for t, a in loads:
            nc.sync.dma_start(t, a)

        # ---- prologue ----
        # cummax along h: log-doubling shifted max, ping-pong buffers.
        cmax = cpool.tile([BL, N_H], F32, tag="cmax")
        cmx2 = cpool.tile([BL, N_H], F32, tag="cmx2")
        cur, nxt = cne, cmx2
        s = 1
        while s < N_H:
            nc.vector.tensor_tensor(nxt[:, s:N_H], cur[:, s:N_H],
                                    cur[:, 0:N_H - s], ALU.max)
            nc.scalar.copy(nxt[:, 0:s], cur[:, 0:s])
            cur, nxt = nxt, (cmax if nxt is cmx2 else cmx2)
            s *= 2
        if cur is not cmax:
            nc.scalar.copy(cmax[:], cur[:])

        # transpose cummax -> [h, b] columns via PE
        cmcol = cpool.tile([128, N_HT * BL], F32, tag="cmcol")
        for ht in range(N_HT):
            tp = ppool.tile([128, BL], F32, tag="tp")
            nc.tensor.transpose(tp[:], cmax[:, ht * 128:(ht + 1) * 128],
                                ceye[:])
            nc.scalar.copy(cmcol[:, ht * BL:(ht + 1) * BL], tp[:])

        # (obs - h_base) * mask, transposed layout [f, b]
        obsmT = cpool.tile([128, 2 * BL], F32, tag="obsmT")
        ot1 = cpool.tile([128, 2 * BL], F32, tag="ot1")
        nc.vector.tensor_scalar(ot1[:], cobst[:], chb[:, 0:1], None,
                                ALU.subtract)
        nc.vector.tensor_tensor(obsmT[:], ot1[:], cmaskt[:], ALU.mult)

        # ---- main loop ----
        hv0 = hvpool.tile([128, BL], F32, tag="hv0")  # f-half 0, columns = b
        hv1 = hvpool.tile([128, BL], F32, tag="hv1")
        for b in range(BL):
            for ht in range(N_HT):
                necol = cnet[:, ht * BL + b: ht * BL + b + 1]
                cmc = cmcol[:, ht * BL + b: ht * BL + b + 1]
                pen = pen_pool.tile([128, N_F], F32, tag="pen")
                nc.vector.tensor_scalar(pen[:], cthr[:], cmc, 1.0e18,
                                        ALU.is_le, ALU.mult)
                rat = rat_pool.tile([128, N_F], F32, tag="rat")
                nc.vector.tensor_scalar(rat[:], cinv[:], necol, None,
                                        ALU.mult)
                rs2 = rs_pool.tile([128, N_F], F32, tag="rs2")
                nc.vector.tensor_tensor(rs2[:], pen[:], rat[:], ALU.subtract)
                mu = mu_pool.tile([128, N_F], F32, tag="mu")
                if MU_PATH == "absrsqrt":
                    nc.scalar.activation(mu[:], rs2[:], AF.Abs_reciprocal_sqrt,
                                         bias=1.0, scale=1.0)
                else:
                    lg = mu_pool.tile([128, N_F], F32, tag="lg")
                    nc.scalar.activation(lg[:], rs2[:], AF.Ln,
                                         bias=1.0, scale=1.0)
                    nc.scalar.activation(mu[:], lg[:], AF.Exp,
                                         bias=0.0, scale=-0.5)
                nc.tensor.matmul(hv0[:, b:b + 1], mu[:, 0:128],
                                 cw[:, ht:ht + 1],
                                 start=(ht == 0), stop=(ht == N_HT - 1))
                nc.tensor.matmul(hv1[:, b:b + 1], mu[:, 128:256],
                                 cw[:, ht:ht + 1],
                                 start=(ht == 0), stop=(ht == N_HT - 1))

        # ---- epilogue ----
        acc = cpool.tile([128, 8], F32, tag="acc")
        nc.vector.memset(acc[:], 0.0)

        # abel: sum((hv - obsm)^2 * mask) cols 0,1; n_valid cols 2,3
        for hf, hv in ((0, hv0), (1, hv1)):
            sl = slice(hf * BL, (hf + 1) * BL)
            d = cpool.tile([128, BL], F32, tag=f"d{hf}")
            nc.vector.tensor_tensor(d[:], hv[:], obsmT[:, sl], ALU.subtract)
            dm = cpool.tile([128, BL], F32, tag=f"dm{hf}")
            nc.vector.tensor_tensor(dm[:], d[:], cmaskt[:, sl], ALU.mult)
            sq = cpool.tile([128, BL], F32, tag=f"sq{hf}")
            nc.scalar.activation(sq[:], dm[:], AF.Square, bias=0.0, scale=1.0,
                                 accum_out=acc[:, hf:hf + 1])
            nc.vector.tensor_reduce(acc[:, 2 + hf:3 + hf], cmaskt[:, sl],
                                    mybir.AxisListType.X, ALU.add)

        # mono: col4 = sum(relu(-dne+c*eps)*asc), col5 = sum(relu(dne+c*eps)),
        #       col6 = sum(relu(dne+c*eps)*asc)
        dne = cpool.tile([BL, N_H - 1], F32, tag="dne")
        nc.vector.tensor_tensor(dne[:], cne[:, 1:N_H], cne[:, 0:N_H - 1],
                                ALU.subtract)
        asc = cpool.tile([BL, N_H - 1], F32, tag="asc")
        nc.vector.tensor_scalar(asc[:], cmax[:, 0:N_H - 1],
                                cmax[:, N_H - 1:N_H], None, ALU.is_lt)
        relu_a = cpool.tile([BL, N_H - 1], F32, tag="relu_a")
        nc.scalar.activation(relu_a[:], dne[:], AF.Relu,
                             bias=ceps_t[0:BL, 0:1], scale=-1.0)
        pa = cpool.tile([BL, N_H - 1], F32, tag="pa")
        nc.vector.tensor_tensor(pa[:], relu_a[:], asc[:], ALU.mult)
        nc.vector.tensor_reduce(acc[0:BL, 4:5], pa[:],
                                mybir.AxisListType.X, ALU.add)
        relu_d = cpool.tile([BL, N_H - 1], F32, tag="relu_d")
        nc.scalar.activation(relu_d[:], dne[:], AF.Relu,
                             bias=ceps_t[0:BL, 0:1], scale=1.0,
                             accum_out=acc[0:BL, 5:6])
        pd = cpool.tile([BL, N_H - 1], F32, tag="pd")
        nc.vector.tensor_tensor(pd[:], relu_d[:], asc[:], ALU.mult)
        nc.vector.tensor_reduce(acc[0:BL, 6:7], pd[:],
                                mybir.AxisListType.X, ALU.add)

        # bg: col7 = sum((ne - iri)^2)
        dbg = cpool.tile([BL, N_H], F32, tag="dbg")
        nc.vector.tensor_tensor(dbg[:], cne[:], ciri[:], ALU.subtract)
        sbg = cpool.tile([BL, N_H], F32, tag="sbg")
        nc.scalar.activation(sbg[:], dbg[:], AF.Square, bias=0.0, scale=1.0,
                             accum_out=acc[0:BL, 7:8])

        nc.sync.dma_start(out, acc[:])

    nc.compile()
    return nc


def _shard_inputs(ne_pred, h_virt_obs, ne_iri, obs_mask, h_grid, f_grid):
    f2 = f_grid.astype(np.float64) ** 2
    den = FP2_CONST * (f2 + 1e-30)
    invf2p = np.tile((1.0 / den).astype(np.float32), (128, 1))
    threshp = np.tile((REFLECT_GUARD * den).astype(np.float32), (128, 1))

    dh = float(h_grid[1]) - float(h_grid[0])
    h_base = float(h_grid[0])
    w = np.full(N_H, dh, np.float64)
    w[0] *= 0.5
    w[-1] *= 0.5
    wmat = w.reshape(N_HT, 128).T.astype(np.float32).copy()
    eye32 = np.eye(BL, dtype=np.float32)
    hbasecol = np.full((128, 1), h_base, np.float32)
    cepscol = np.full((128, 1), FP2_CONST * EPS_MONO, np.float32)
    maskf = obs_mask.astype(np.float32)

    in_maps = []
    for k in range(N_CORES):
        sl = slice(k * BL, (k + 1) * BL)
        ne = np.ascontiguousarray(ne_pred[sl])
        in_maps.append({
            "ne": ne,
            "ne_t": np.ascontiguousarray(ne.T),
            "iri": np.ascontiguousarray(ne_iri[sl]),
            "obs_t": np.ascontiguousarray(h_virt_obs[sl].T),
            "mask_t": np.ascontiguousarray(maskf[sl].T),
            "invf2p": invf2p, "threshp": threshp, "wmat": wmat,
            "eye32": eye32, "hbasecol": hbasecol, "cepscol": cepscol,
        })
    return in_maps


def _combine(outs):
    s_abel = nv = asc_s = rd_s = rdasc_s = bg_s = 0.0
    for o in outs:
        o = o.astype(np.float64)
        s_abel += o[:, 0].sum() + o[:, 1].sum()
        nv += o[:, 2].sum() + o[:, 3].sum()
        asc_s += o[0:BL, 4].sum()
        rd_s += o[0:BL, 5].sum()
        rdasc_s += o[0:BL, 6].sum()
        bg_s += o[0:BL, 7].sum()
    l_abel = s_abel / max(nv, 1.0)
    l_mono = (asc_s + rd_s - rdasc_s) / (FP2_CONST * B * (N_H - 1))
    l_bg = bg_s / (B * N_H)
    total = LAMBDA_PHY * l_abel + LAMBDA_MONO * l_mono + LAMBDA_BG * l_bg
    return np.array([total, l_abel, l_mono, l_bg], dtype=np.float32)


def _ensure_ntff_hook():
    """Register the axon NTFF profile hook if the image's antenv lacks it."""
    try:
        import antenv.axon_hooks  # noqa: F401
        return
    except ImportError:
        pass
    import sys
    import types
    try:
        import antenv
        from trn_agent_boot.trn_boot import _ntff_profile_via_ctypes
        m = types.ModuleType("antenv.axon_hooks")
        state = {"h": None}
        m.set_axon_ntff_profile_hook = lambda h: state.__setitem__("h", h)
        m.get_axon_ntff_profile_hook = lambda: state["h"]
        sys.modules["antenv.axon_hooks"] = m
        antenv.axon_hooks = m
        hook = _ntff_profile_via_ctypes("/opt/axon/libaxon_pjrt.so")
        m.set_axon_ntff_profile_hook(hook)
    except Exception:
        pass


def kernel(ne_pred, h_virt_obs, ne_iri, obs_mask, h_grid, f_grid,
           **run_kwargs):
    if run_kwargs.get("trace"):
        _ensure_ntff_hook()
    if "nc" not in _cache:
        _cache["nc"] = _build_program()
    nc = _cache["nc"]
    in_maps = _shard_inputs(ne_pred, h_virt_obs, ne_iri, obs_mask,
                            h_grid, f_grid)
    res = run_bass_kernel_spmd(nc, in_maps, list(range(N_CORES)),
                               **run_kwargs)
    outs = [r["out"] for r in res.results]
    result = _combine(outs)
    if run_kwargs:
        return result, res
    return result
